# revision 3
# baseline (speedup 1.0000x reference)
"""Binary-weight 3x3 SAME conv (NHWC) on Trainium2, data-parallel over 8 cores.

Problem: x (32,56,56,256) f32, w (3,3,256,256) f32.
  out = conv2d(x, sign(clip(w,-1,1)), SAME, stride 1)   # NHWC / HWIO

Strategy (per core, 4 images), fp8 hi+lo with DoubleRowSwInterleave matmuls:
  - x is split per element as x ~ hi + lo with hi = e4m3(bf16(x)) and
    lo = e4m3(bf16(x) - hi); products with the +-1 weights are then accurate
    to ~bf16 level while the PE runs fp8 DoubleRow matmuls, measured at
    ~61 ns per K=256/N=448 instruction in SwInterleave mode (vs 228 ns for
    the bf16 K=128 equivalent pair).
  - DMA x tiles [112pos, 256ci] f32 -> bf16 (DVE) -> TensorE-transpose to
    channel-major psum; DVE then writes the hi plane (quantize) and lo plane
    (subtract + quantize) as zero-padded 58x58 fp8 planes per image, so SAME
    padding becomes plain reads.
  - Conv accumulates 18 DRSW matmuls (9 taps x {hi,lo}) per psum tile of
    8 output rows: psum[128co, 448] += sum_cc s[tap][ci,co].T @ plane[...]
    with both cc chunks folded into one DoubleRow instruction.
  - DRSW weights are stored interleaved: raw[k, 2j+i] = W_i[k, 127-j]; the
    column reversal is absorbed by flipping the co axis on the host.
  - Output DMAs straight from PSUM to HBM, channel-major (2,128co,4b,3136pos)
    f32; host transposes and un-flips.

Built with bacc.Bacc + nc.compile(): walrus allows only one sync wait per
instruction, and Bacc's move_matmul_waits_to_ldweights/generate_event_semaphores
passes enforce that.
"""

import numpy as np

import concourse.bacc as bacc
import concourse.mybir as mybir
import concourse.tile as tile

# ---- problem constants (hardcoded; kernel.py must be self-contained) ----
B_FULL, H, W, CI, CO, K = 32, 56, 56, 256, 256, 3
N_CORES = 8
B = B_FULL // N_CORES          # 4 images per core
IMG = H * W                    # 3136 valid positions per image
P = 128
HP, WP = H + 2, W + 2          # 58x58 zero-padded plane per image
IMGP = HP * WP                 # 3364
POSP = B * IMGP                # 13456 padded positions per core
TROWS = 2                      # image rows per transpose tile
TPOS = TROWS * W               # 112 positions per transpose tile
NT_IMG = H // TROWS            # 28 transpose tiles per image
CI_C = CI // P                 # 2 contraction chunks
CO_C = CO // P                 # 2 output-channel chunks
YCHUNK = 8                     # output rows per psum tile
NCHUNK = H // YCHUNK           # 7 chunks per image
FREE = YCHUNK * W              # 448 <= 512 psum fp32 bank limit
KK = K * K                     # 9 taps

F32 = mybir.dt.float32
BF16 = mybir.dt.bfloat16
FP8 = mybir.dt.float8e4

DRSW = mybir.MatmulPerfMode.DoubleRowSwInterleave


def _emit_body(nc, pools, x_d, w_d, o_d):
    import ml_dtypes

    (const_pool, ws_pool, win_pool, xin_pool, xc_pool, xt_pool, out_pool,
     tpsum_pool, cpsum_pool) = pools

    x_flat = x_d.ap().flatten_outer_dims()      # [B*IMG, CI]

    # identity via inline const (keeps gpsimd out of the program); bf16 so
    # transposes run at 1 cycle/row on the PE.
    ident_dram = nc.inline_tensor(np.eye(P, dtype=ml_dtypes.bfloat16), name="ident_c")
    ident = const_pool.tile([P, P], BF16, name="ident")
    nc.sync.dma_start(out=ident, in_=ident_dram.ap())

    # ---- binarize weights into DRSW-interleaved fp8: s_all[128ci, 9t, 2oc, 256raw]
    # raw[k, t, oc, 2j+i] = sign(w)[t, cc=i, k, oc*128 + j]; the DRSW column
    # reversal is left in and undone on the host by flipping co per oc chunk.
    # The w DMAs + signs are EMITTED after the first activation tiles (below):
    # finely split pieces + address-level deps let the first conv group's
    # matmuls consume sign pieces just-in-time as they land.
    w_src = w_d.ap().rearrange("ky kx (cc p) co -> p (ky kx cc) co", p=P)
    wtile = win_pool.tile([P, KK * CI_C, CO], F32, name="wtile")
    s_all = ws_pool.tile([P, KK, CO_C, 2 * P], FP8, name="s_all")
    s_wview = s_all.rearrange("p t o (j i) -> p t o j i", i=2)
    w_bounds = [0, 3, 6, 9, 12, 15, 18]

    def emit_weights():
        for a, bnd in zip(w_bounds[:-1], w_bounds[1:]):
            nc.sync.dma_start(out=wtile[:, a:bnd], in_=w_src[:, a:bnd])
        for a, bnd in zip(w_bounds[:-1], w_bounds[1:]):
            for u in range(a, bnd):
                t, i = divmod(u, CI_C)
                sv = s_wview[:, t, :, :, i]          # [p, 2oc, 128j]
                src = wtile[:, u, :].rearrange("p (o j) -> p o j", j=P)
                # sign(w) = 2*(w >= 0) - 1 (exact +-1 in e4m3); on DVE so
                # conv matmuls only wait on the DVE semaphore.
                nc.vector.tensor_scalar(sv, src, 0.0, None, mybir.AluOpType.is_ge)
                nc.vector.tensor_scalar(
                    sv, sv, 2.0, -1.0, mybir.AluOpType.mult, mybir.AluOpType.add,
                )

    # ---- channel-major activations: fp8 hi + lo zero-padded 58x58 planes ----
    xt_hi = xt_pool.tile([P, CI_C, POSP], FP8, name="xt_hi")
    xt_lo = xt_pool.tile([P, CI_C, POSP], FP8, name="xt_lo")
    hi_plane = xt_hi.rearrange("p c (b y x) -> p c b y x", y=HP, x=WP)
    lo_plane = xt_lo.rearrange("p c (b y x) -> p c b y x", y=HP, x=WP)

    # zero only the pad strips (top/bottom rows, left/right cols); gpsimd is
    # otherwise idle so this costs nothing on the critical path
    for plane in (hi_plane, lo_plane):
        for b in range(B):
            for cc in range(CI_C):
                nc.gpsimd.memset(plane[:, cc, b, 0, :], 0.0)
                nc.gpsimd.memset(plane[:, cc, b, HP - 1, :], 0.0)
                nc.gpsimd.memset(plane[:, cc, b, 1 : HP - 1, 0], 0.0)
                nc.gpsimd.memset(plane[:, cc, b, 1 : HP - 1, WP - 1], 0.0)

    N_TILES = B * NT_IMG
    emitted = [0]

    def emit_transposes(upto):
        for g in range(emitted[0], min(N_TILES, upto)):
            b, t = divmod(g, NT_IMG)
            xin = xin_pool.tile([TPOS, CI], F32, name="xin", tag="xin")
            src0 = b * IMG + t * TPOS
            nc.sync.dma_start(out=xin, in_=x_flat[src0 : src0 + TPOS, :])
            xc = xc_pool.tile([TPOS, CI], BF16, name="xc", tag="xc")
            nc.vector.tensor_copy(out=xc, in_=xin)
            r0 = t * TROWS + 1  # padded row of first element
            for cc in range(CI_C):
                tps = tpsum_pool.tile([P, TPOS], BF16, name="tps", tag="tps")
                nc.tensor.transpose(
                    tps, xc[:, cc * P : (cc + 1) * P], ident[:TPOS, :TPOS]
                )
                tview = tps.rearrange("p (r x) -> p r x", x=W)
                hv = hi_plane[:, cc, b, r0 : r0 + TROWS, 1 : 1 + W]
                lv = lo_plane[:, cc, b, r0 : r0 + TROWS, 1 : 1 + W]
                # hi = e4m3(bf16 x) on ScalarE; lo = bf16 x - hi on DVE
                nc.scalar.activation(hv, tview, mybir.ActivationFunctionType.Copy)
                nc.vector.tensor_sub(lv, tview, hv)
        emitted[0] = max(emitted[0], min(N_TILES, upto))

    LOOKAHEAD = 5  # transpose tiles emitted ahead of the consuming chunk

    # First activation tiles go ahead of the weight load on the DMA pipe and
    # the DVE queue, so the PE transposes while the weights stream in.
    emit_transposes(5)
    emit_weights()

    for b in range(B):
        hiv = xt_hi[:, :, b * IMGP : (b + 1) * IMGP].rearrange(
            "p c (y x) -> p c y x", x=WP)
        lov = xt_lo[:, :, b * IMGP : (b + 1) * IMGP].rearrange(
            "p c (y x) -> p c y x", x=WP)
        for c in range(NCHUNK):
            y0 = c * YCHUNK
            # conv chunk c reads padded rows [y0, y0+10) = valid rows
            # [y0-1, y0+8] -> needs image tiles t < (y0+10)//2
            need = b * NT_IMG + min(NT_IMG, (y0 + YCHUNK + 2 + 1) // TROWS)
            emit_transposes(need + LOOKAHEAD)
            for oc in range(CO_C):
                cps = cpsum_pool.tile([P, FREE], F32, name="cps", tag="cps")
                n = 0
                for t in range(KK):
                    ky, kx = divmod(t, K)
                    lhs = s_all[:, t, oc, :]
                    for pl in (hiv, lov):
                        rhs = pl[:, :, y0 + ky : y0 + ky + YCHUNK, kx : kx + W]
                        nc.tensor.matmul(
                            cps, lhs, rhs,
                            start=(n == 0), stop=(n == 2 * KK - 1),
                            perf_mode=DRSW,
                        )
                        n += 1
                ot = out_pool.tile([P, FREE], F32, name="ot", tag="ot")
                # psum->sbuf copy on ScalarE, keeping DVE free for quantize
                nc.scalar.activation(ot, cps, mybir.ActivationFunctionType.Copy)
                nc.sync.dma_start(
                    out=o_d.ap()[oc, :, b, y0 * W : (y0 + YCHUNK) * W],
                    in_=ot,
                )


def build_program(reps: int = 1):
    # Bacc (not plain Bass): compile() runs move_matmul_waits_to_ldweights +
    # generate_event_semaphores, required because walrus allows only one sync
    # wait per instruction.
    nc = bacc.Bacc("TRN2", debug=False, num_devices=N_CORES)
    x_d = nc.dram_tensor("x", [B, H, W, CI], F32, kind="ExternalInput")
    w_d = nc.dram_tensor("w", [K, K, CI, CO], F32, kind="ExternalInput")
    o_d = nc.dram_tensor("out", [CO_C, P, B, IMG], F32, kind="ExternalOutput")

    with tile.TileContext(nc) as tc:
        with (
            tc.tile_pool(name="const", bufs=1) as const_pool,
            tc.tile_pool(name="ws", bufs=1) as ws_pool,
            tc.tile_pool(name="win", bufs=1) as win_pool,
            tc.tile_pool(name="xin", bufs=12) as xin_pool,
            tc.tile_pool(name="xcp", bufs=12) as xc_pool,
            tc.tile_pool(name="xtp", bufs=1) as xt_pool,
            tc.tile_pool(name="outs", bufs=4) as out_pool,
            tc.tile_pool(name="tpsum", bufs=3, space="PSUM") as tpsum_pool,
            tc.tile_pool(name="cpsum", bufs=5, space="PSUM") as cpsum_pool,
        ):
            pools = (const_pool, ws_pool, win_pool, xin_pool, xc_pool,
                     xt_pool, out_pool, tpsum_pool, cpsum_pool)
            if reps == 1:
                _emit_body(nc, pools, x_d, w_d, o_d)
            else:
                with tc.For_i(0, reps, 1):
                    _emit_body(nc, pools, x_d, w_d, o_d)
    nc.compile()
    return nc


_NC_CACHE = {}


def _get_program(reps: int = 1):
    if reps not in _NC_CACHE:
        _NC_CACHE[reps] = build_program(reps)
    return _NC_CACHE[reps]


def kernel(x: np.ndarray, w: np.ndarray) -> np.ndarray:
    from concourse.bass_utils import run_bass_kernel_spmd

    x = np.ascontiguousarray(x, dtype=np.float32)
    w = np.ascontiguousarray(w, dtype=np.float32)
    nc = _get_program()
    in_maps = [
        {"x": np.ascontiguousarray(x[c * B : (c + 1) * B]), "w": w}
        for c in range(N_CORES)
    ]
    res = run_bass_kernel_spmd(nc, in_maps, core_ids=list(range(N_CORES))).results
    outs = []
    for c in range(N_CORES):
        r = res[c]["out"]  # (CO_C, P, B, IMG)
        r = r[:, ::-1]     # undo the DRSW column reversal within each oc chunk
        o = r.transpose(2, 3, 0, 1).reshape(B, H, W, CO)
        outs.append(o)
    return np.ascontiguousarray(np.concatenate(outs, axis=0))


# revision 6
# speedup vs baseline: 3.6711x; 3.6711x over previous
"""Binary-weight 3x3 SAME conv (NHWC) on Trainium2, data-parallel over 8 cores.

Problem: x (32,56,56,256) f32, w (3,3,256,256) f32.
  out = conv2d(x, sign(clip(w,-1,1)), SAME, stride 1)   # NHWC / HWIO

Strategy (per core, 4 images), fp8 hi+lo with DoubleRowSwInterleave matmuls:
  - x is split per element as x ~ hi + lo with hi = e4m3(bf16(x)) and
    lo = e4m3(bf16(x) - hi); products with the +-1 weights are then accurate
    to ~bf16 level while the PE runs fp8 DoubleRow matmuls, measured at
    ~61 ns per K=256/N=448 instruction in SwInterleave mode (vs 228 ns for
    the bf16 K=128 equivalent pair).
  - DMA x tiles [112pos, 256ci] f32 -> bf16 (DVE) -> TensorE-transpose to
    channel-major psum; DVE then writes the hi plane (quantize) and lo plane
    (subtract + quantize) as zero-padded 58x58 fp8 planes per image, so SAME
    padding becomes plain reads.
  - Conv accumulates 18 DRSW matmuls (9 taps x {hi,lo}) per psum tile of
    8 output rows: psum[128co, 448] += sum_cc s[tap][ci,co].T @ plane[...]
    with both cc chunks folded into one DoubleRow instruction.
  - DRSW weights are stored interleaved: raw[k, 2j+i] = W_i[k, 127-j]; the
    column reversal is absorbed by flipping the co axis on the host.
  - Output DMAs straight from PSUM to HBM, channel-major (2,128co,4b,3136pos)
    f32; host transposes and un-flips.

Built with bacc.Bacc + nc.compile(): walrus allows only one sync wait per
instruction, and Bacc's move_matmul_waits_to_ldweights/generate_event_semaphores
passes enforce that.
"""

import numpy as np

import concourse.bacc as bacc
import concourse.mybir as mybir
import concourse.tile as tile

# ---- problem constants (hardcoded; kernel.py must be self-contained) ----
B_FULL, H, W, CI, CO, K = 32, 56, 56, 256, 256, 3
N_CORES = 8
B = B_FULL // N_CORES          # 4 images per core
IMG = H * W                    # 3136 valid positions per image
P = 128
HP, WP = H + 2, W + 2          # 58x58 zero-padded plane per image
IMGP = HP * WP                 # 3364
POSP = B * IMGP                # 13456 padded positions per core
TROWS = 2                      # image rows per transpose tile
TPOS = TROWS * W               # 112 positions per transpose tile
NT_IMG = H // TROWS            # 28 transpose tiles per image
CI_C = CI // P                 # 2 contraction chunks
CO_C = CO // P                 # 2 output-channel chunks
YCHUNK = 8                     # output rows per psum tile
NCHUNK = H // YCHUNK           # 7 chunks per image
FREE = YCHUNK * W              # 448 <= 512 psum fp32 bank limit
KK = K * K                     # 9 taps
K_LO = 9                       # taps getting the lo-residual correction

F32 = mybir.dt.float32
BF16 = mybir.dt.bfloat16
FP8 = mybir.dt.float8e4

DRSW = mybir.MatmulPerfMode.DoubleRowSwInterleave


def _emit_body(nc, pools, x_d, w_d, o_d):
    import ml_dtypes

    (const_pool, ws_pool, win_pool, xin_pool, xc_pool, xt_pool, out_pool,
     tpsum_pool, cpsum_pool) = pools

    x_flat = x_d.ap().flatten_outer_dims()      # [B*IMG, CI]

    # identity via inline const (keeps gpsimd out of the program); bf16 so
    # transposes run at 1 cycle/row on the PE.
    ident_dram = nc.inline_tensor(np.eye(P, dtype=ml_dtypes.bfloat16), name="ident_c")
    ident = const_pool.tile([P, P], BF16, name="ident")
    nc.sync.dma_start(out=ident, in_=ident_dram.ap())

    # ---- binarize weights into DRSW-interleaved fp8: s_all[128ci, 9t, 2oc, 256raw]
    # raw[k, t, oc, 2j+i] = sign(w)[t, cc=i, k, oc*128 + j]; the DRSW column
    # reversal is left in and undone on the host by flipping co per oc chunk.
    # The w DMAs + signs are EMITTED after the first activation tiles (below):
    # finely split pieces + address-level deps let the first conv group's
    # matmuls consume sign pieces just-in-time as they land.
    w_src = w_d.ap().rearrange("ky kx (cc p) co -> p (ky kx cc) co", p=P)
    wtile = win_pool.tile([P, KK * CI_C, CO], F32, name="wtile")
    s_all = ws_pool.tile([P, KK, CO_C, 2 * P], FP8, name="s_all")
    s_wview = s_all.rearrange("p t o (j i) -> p t o j i", i=2)
    w_bounds = [0, 3, 6, 9, 12, 15, 18]

    def emit_weights():
        for a, bnd in zip(w_bounds[:-1], w_bounds[1:]):
            nc.sync.dma_start(out=wtile[:, a:bnd], in_=w_src[:, a:bnd])
        for a, bnd in zip(w_bounds[:-1], w_bounds[1:]):
            for u in range(a, bnd):
                t, i = divmod(u, CI_C)
                sv = s_wview[:, t, :, :, i]          # [p, 2oc, 128j]
                src = wtile[:, u, :].rearrange("p (o j) -> p o j", j=P)
                # sign(w) = 2*(w >= 0) - 1 (exact +-1 in e4m3); on DVE so
                # conv matmuls only wait on the DVE semaphore.
                nc.vector.tensor_scalar(sv, src, 0.0, None, mybir.AluOpType.is_ge)
                nc.vector.tensor_scalar(
                    sv, sv, 2.0, -1.0, mybir.AluOpType.mult, mybir.AluOpType.add,
                )

    # ---- channel-major activations: fp8 hi + lo zero-padded 58x58 planes ----
    xt_hi = xt_pool.tile([P, CI_C, POSP], FP8, name="xt_hi")
    xt_lo = xt_pool.tile([P, CI_C, POSP], FP8, name="xt_lo")
    hi_plane = xt_hi.rearrange("p c (b y x) -> p c b y x", y=HP, x=WP)
    lo_plane = xt_lo.rearrange("p c (b y x) -> p c b y x", y=HP, x=WP)

    # zero only the pad strips (top/bottom rows, left/right cols); gpsimd is
    # otherwise idle so this costs nothing on the critical path
    for plane in (hi_plane, lo_plane):
        for b in range(B):
            for cc in range(CI_C):
                nc.gpsimd.memset(plane[:, cc, b, 0, :], 0.0)
                nc.gpsimd.memset(plane[:, cc, b, HP - 1, :], 0.0)
                nc.gpsimd.memset(plane[:, cc, b, 1 : HP - 1, 0], 0.0)
                nc.gpsimd.memset(plane[:, cc, b, 1 : HP - 1, WP - 1], 0.0)

    # 16 transpose psum slots in 2 banks: [P, bank, slot, 128] bf16, 112 used
    tslots = tpsum_pool.tile([P, 2, 8, P], BF16, name="tslots")

    N_TILES = B * NT_IMG
    emitted = [0]

    def emit_transposes(upto):
        for g in range(emitted[0], min(N_TILES, upto)):
            b, t = divmod(g, NT_IMG)
            xin = xin_pool.tile([TPOS, CI], F32, name="xin", tag="xin")
            src0 = b * IMG + t * TPOS
            nc.sync.dma_start(out=xin, in_=x_flat[src0 : src0 + TPOS, :])
            xc = xc_pool.tile([TPOS, CI], BF16, name="xc", tag="xc")
            # cast on ScalarE: its queue holds only casts, so it never blocks
            nc.scalar.activation(xc, xin, mybir.ActivationFunctionType.Copy)
            r0 = t * TROWS + 1  # padded row of first element
            for cc in range(CI_C):
                sl = (2 * g + cc) % 16
                tps = tslots[:, sl // 8, sl % 8, :TPOS]
                nc.tensor.transpose(
                    tps, xc[:, cc * P : (cc + 1) * P], ident[:TPOS, :TPOS]
                )
                tview = tps.rearrange("p (r x) -> p r x", x=W)
                hv = hi_plane[:, cc, b, r0 : r0 + TROWS, 1 : 1 + W]
                lv = lo_plane[:, cc, b, r0 : r0 + TROWS, 1 : 1 + W]
                # hi = e4m3(bf16 x), lo = bf16 x - hi, back-to-back on DVE
                nc.vector.tensor_copy(out=hv, in_=tview)
                nc.vector.tensor_sub(lv, tview, hv)
        emitted[0] = max(emitted[0], min(N_TILES, upto))

    LOOKAHEAD = 8  # transpose tiles emitted ahead of the consuming chunk

    # First activation tiles go ahead of the weight load on the DMA pipe and
    # the DVE queue, so the PE transposes while the weights stream in.
    emit_transposes(8)
    emit_weights()

    for b in range(B):
        hiv = xt_hi[:, :, b * IMGP : (b + 1) * IMGP].rearrange(
            "p c (y x) -> p c y x", x=WP)
        lov = xt_lo[:, :, b * IMGP : (b + 1) * IMGP].rearrange(
            "p c (y x) -> p c y x", x=WP)
        for c in range(NCHUNK):
            y0 = c * YCHUNK
            # conv chunk c reads padded rows [y0, y0+10) = valid rows
            # [y0-1, y0+8] -> needs image tiles t < (y0+10)//2
            need = b * NT_IMG + min(NT_IMG, (y0 + YCHUNK + 2 + 1) // TROWS)
            emit_transposes(need + LOOKAHEAD)
            for oc in range(CO_C):
                cps = cpsum_pool.tile([P, FREE], F32, name="cps", tag="cps")
                n = 0
                for t in range(KK):
                    ky, kx = divmod(t, K)
                    lhs = s_all[:, t, oc, :]
                    planes = (hiv, lov) if t < K_LO else (hiv,)
                    for pl in planes:
                        rhs = pl[:, :, y0 + ky : y0 + ky + YCHUNK, kx : kx + W]
                        nc.tensor.matmul(
                            cps, lhs, rhs,
                            start=(n == 0), stop=(n == KK + K_LO - 1),
                            perf_mode=DRSW,
                        )
                        n += 1
                ot = out_pool.tile([P, FREE], F32, name="ot", tag="ot")
                # psum->sbuf copy on ScalarE (gpsimd cannot read PSUM); DVE
                # stays on the hi/lo quant path
                nc.scalar.activation(ot, cps, mybir.ActivationFunctionType.Copy)
                nc.sync.dma_start(
                    out=o_d.ap()[oc, :, b, y0 * W : (y0 + YCHUNK) * W],
                    in_=ot,
                )


def build_program(reps: int = 1):
    # Bacc (not plain Bass): compile() runs move_matmul_waits_to_ldweights +
    # generate_event_semaphores, required because walrus allows only one sync
    # wait per instruction.
    nc = bacc.Bacc("TRN2", debug=False, num_devices=N_CORES)
    x_d = nc.dram_tensor("x", [B, H, W, CI], F32, kind="ExternalInput")
    w_d = nc.dram_tensor("w", [K, K, CI, CO], F32, kind="ExternalInput")
    o_d = nc.dram_tensor("out", [CO_C, P, B, IMG], F32, kind="ExternalOutput")

    with tile.TileContext(nc) as tc:
        with (
            tc.tile_pool(name="const", bufs=1) as const_pool,
            tc.tile_pool(name="ws", bufs=1) as ws_pool,
            tc.tile_pool(name="win", bufs=1) as win_pool,
            tc.tile_pool(name="xin", bufs=12) as xin_pool,
            tc.tile_pool(name="xcp", bufs=12) as xc_pool,
            tc.tile_pool(name="xtp", bufs=1) as xt_pool,
            tc.tile_pool(name="outs", bufs=4) as out_pool,
            tc.tile_pool(name="tpsum", bufs=1, space="PSUM") as tpsum_pool,
            tc.tile_pool(name="cpsum", bufs=6, space="PSUM") as cpsum_pool,
        ):
            pools = (const_pool, ws_pool, win_pool, xin_pool, xc_pool,
                     xt_pool, out_pool, tpsum_pool, cpsum_pool)
            if reps == 1:
                _emit_body(nc, pools, x_d, w_d, o_d)
            else:
                with tc.For_i(0, reps, 1):
                    _emit_body(nc, pools, x_d, w_d, o_d)
    nc.compile()
    return nc


_NC_CACHE = {}


def _get_program(reps: int = 1):
    if reps not in _NC_CACHE:
        _NC_CACHE[reps] = build_program(reps)
    return _NC_CACHE[reps]


def kernel(x: np.ndarray, w: np.ndarray) -> np.ndarray:
    from concourse.bass_utils import run_bass_kernel_spmd

    x = np.ascontiguousarray(x, dtype=np.float32)
    w = np.ascontiguousarray(w, dtype=np.float32)
    nc = _get_program()
    in_maps = [
        {"x": np.ascontiguousarray(x[c * B : (c + 1) * B]), "w": w}
        for c in range(N_CORES)
    ]
    res = run_bass_kernel_spmd(nc, in_maps, core_ids=list(range(N_CORES))).results
    outs = []
    for c in range(N_CORES):
        r = res[c]["out"]  # (CO_C, P, B, IMG)
        r = r[:, ::-1]     # undo the DRSW column reversal within each oc chunk
        o = r.transpose(2, 3, 0, 1).reshape(B, H, W, CO)
        outs.append(o)
    return np.ascontiguousarray(np.concatenate(outs, axis=0))


# revision 8
# speedup vs baseline: 5.1107x; 1.3921x over previous
"""Binary-weight 3x3 SAME conv (NHWC) on Trainium2, data-parallel over 8 cores.

Problem: x (32,56,56,256) f32, w (3,3,256,256) f32.
  out = conv2d(x, sign(clip(w,-1,1)), SAME, stride 1)   # NHWC / HWIO

Strategy (per core, 4 images), fp8 hi+lo with DoubleRowSwInterleave matmuls:
  - x is split per element as x ~ hi + lo with hi = e4m3(bf16(x)) and
    lo = e4m3(bf16(x) - hi); products with the +-1 weights are then accurate
    to ~bf16 level while the PE runs fp8 DoubleRow matmuls, measured at
    ~61 ns per K=256/N=448 instruction in SwInterleave mode (vs 228 ns for
    the bf16 K=128 equivalent pair).
  - DMA x tiles [112pos, 256ci] f32 -> bf16 (DVE) -> TensorE-transpose to
    channel-major psum; DVE then writes the hi plane (quantize) and lo plane
    (subtract + quantize) as zero-padded 58x58 fp8 planes per image, so SAME
    padding becomes plain reads.
  - Conv accumulates 18 DRSW matmuls (9 taps x {hi,lo}) per psum tile of
    8 output rows: psum[128co, 448] += sum_cc s[tap][ci,co].T @ plane[...]
    with both cc chunks folded into one DoubleRow instruction.
  - DRSW weights are stored interleaved: raw[k, 2j+i] = W_i[k, 127-j]; the
    column reversal is absorbed by flipping the co axis on the host.
  - Output DMAs straight from PSUM to HBM, channel-major (2,128co,4b,3136pos)
    f32; host transposes and un-flips.

Built with bacc.Bacc + nc.compile(): walrus allows only one sync wait per
instruction, and Bacc's move_matmul_waits_to_ldweights/generate_event_semaphores
passes enforce that.
"""

import numpy as np

import concourse.bacc as bacc
import concourse.mybir as mybir
import concourse.tile as tile

# ---- problem constants (hardcoded; kernel.py must be self-contained) ----
B_FULL, H, W, CI, CO, K = 32, 56, 56, 256, 256, 3
N_CORES = 8
B = B_FULL // N_CORES          # 4 images per core
IMG = H * W                    # 3136 valid positions per image
P = 128
HP, WP = H + 2, W + 2          # 58x58 zero-padded plane per image
IMGP = HP * WP                 # 3364
POSP = B * IMGP                # 13456 padded positions per core
TROWS = 2                      # image rows per transpose tile
TPOS = TROWS * W               # 112 positions per transpose tile
NT_IMG = H // TROWS            # 28 transpose tiles per image
CI_C = CI // P                 # 2 contraction chunks
CO_C = CO // P                 # 2 output-channel chunks
YCHUNK = 8                     # output rows per psum tile
NCHUNK = H // YCHUNK           # 7 chunks per image
FREE = YCHUNK * W              # 448 <= 512 psum fp32 bank limit
KK = K * K                     # 9 taps
K_LO = 9                       # taps getting the lo-residual correction

# pipeline knobs (sim-swept): engine per stage, psum slot layout, lookahead
CFG = dict(
    tslot="pool3",      # "pack16" one 2-bank tile | "pool3" tile-per-transpose
    cast="scalar",      # engine for xin f32->bf16 cast
    hi="vector",        # engine for hi quantize (psum->fp8 plane)
    lo="vector",        # engine for lo subtract (always DVE-capable)
    out="scalar",       # engine for conv psum->sbuf copy
    lookahead=8,
    cpsum=5,
    xgroup=4,           # transpose tiles per xin DMA (bigger = fewer DMAs)
    outq="act",         # DMA queue for output: "act" (Activation) | "sp"
)

F32 = mybir.dt.float32
BF16 = mybir.dt.bfloat16
FP8 = mybir.dt.float8e4

DRSW = mybir.MatmulPerfMode.DoubleRowSwInterleave


def _emit_body(nc, pools, x_d, w_d, o_d):
    import ml_dtypes

    (const_pool, ws_pool, win_pool, xin_pool, xc_pool, xt_pool, out_pool,
     tpsum_pool, cpsum_pool) = pools

    x_flat = x_d.ap().flatten_outer_dims()      # [B*IMG, CI]

    # identity via inline const (keeps gpsimd out of the program); bf16 so
    # transposes run at 1 cycle/row on the PE.
    ident_dram = nc.inline_tensor(np.eye(P, dtype=ml_dtypes.bfloat16), name="ident_c")
    ident = const_pool.tile([P, P], BF16, name="ident")
    nc.sync.dma_start(out=ident, in_=ident_dram.ap())

    # ---- binarize weights into DRSW-interleaved fp8: s_all[128ci, 9t, 2oc, 256raw]
    # raw[k, t, oc, 2j+i] = sign(w)[t, cc=i, k, oc*128 + j]; the DRSW column
    # reversal is left in and undone on the host by flipping co per oc chunk.
    # The w DMAs + signs are EMITTED after the first activation tiles (below):
    # finely split pieces + address-level deps let the first conv group's
    # matmuls consume sign pieces just-in-time as they land.
    w_src = w_d.ap().rearrange("ky kx (cc p) co -> p (ky kx cc) co", p=P)
    wtile = win_pool.tile([P, KK * CI_C, CO], F32, name="wtile")
    s_all = ws_pool.tile([P, KK, CO_C, 2 * P], FP8, name="s_all")
    s_wview = s_all.rearrange("p t o (j i) -> p t o j i", i=2)
    w_bounds = [0, 3, 6, 9, 12, 15, 18]

    def emit_weights():
        for a, bnd in zip(w_bounds[:-1], w_bounds[1:]):
            nc.sync.dma_start(out=wtile[:, a:bnd], in_=w_src[:, a:bnd])
        for a, bnd in zip(w_bounds[:-1], w_bounds[1:]):
            for u in range(a, bnd):
                t, i = divmod(u, CI_C)
                sv = s_wview[:, t, :, :, i]          # [p, 2oc, 128j]
                src = wtile[:, u, :].rearrange("p (o j) -> p o j", j=P)
                # sign(w) = 2*(w >= 0) - 1 (exact +-1 in e4m3); on DVE so
                # conv matmuls only wait on the DVE semaphore.
                nc.vector.tensor_scalar(sv, src, 0.0, None, mybir.AluOpType.is_ge)
                nc.vector.tensor_scalar(
                    sv, sv, 2.0, -1.0, mybir.AluOpType.mult, mybir.AluOpType.add,
                )

    # ---- channel-major activations: fp8 hi + lo zero-padded 58x58 planes ----
    xt_hi = xt_pool.tile([P, CI_C, POSP], FP8, name="xt_hi")
    xt_lo = xt_pool.tile([P, CI_C, POSP], FP8, name="xt_lo")
    hi_plane = xt_hi.rearrange("p c (b y x) -> p c b y x", y=HP, x=WP)
    lo_plane = xt_lo.rearrange("p c (b y x) -> p c b y x", y=HP, x=WP)

    # zero only the pad strips (top/bottom rows, left/right cols); gpsimd is
    # otherwise idle so this costs nothing on the critical path
    for plane in (hi_plane, lo_plane):
        for b in range(B):
            for cc in range(CI_C):
                nc.gpsimd.memset(plane[:, cc, b, 0, :], 0.0)
                nc.gpsimd.memset(plane[:, cc, b, HP - 1, :], 0.0)
                nc.gpsimd.memset(plane[:, cc, b, 1 : HP - 1, 0], 0.0)
                nc.gpsimd.memset(plane[:, cc, b, 1 : HP - 1, WP - 1], 0.0)

    def eng(name):
        return {"vector": nc.vector, "scalar": None, "gpsimd": nc.gpsimd}[name]

    if CFG["tslot"] == "pack16":
        # 16 transpose psum slots in 2 banks: [P, bank, slot, 128] bf16
        tslots = tpsum_pool.tile([P, 2, 8, P], BF16, name="tslots")

        def tslot(g, cc):
            sl = (2 * g + cc) % 16
            return tslots[:, sl // 8, sl % 8, :TPOS]
    else:
        def tslot(g, cc):
            return tpsum_pool.tile([P, TPOS], BF16, name="tps", tag="tps")

    def copy_op(engine_name, out, in_):
        if engine_name == "scalar":
            nc.scalar.activation(out, in_, mybir.ActivationFunctionType.Copy)
        else:
            eng(engine_name).tensor_copy(out=out, in_=in_)

    N_TILES = B * NT_IMG
    emitted = [0]

    XG = CFG["xgroup"]
    xin_groups = {}

    def emit_transposes(upto):
        for g in range(emitted[0], min(N_TILES, upto)):
            b, t = divmod(g, NT_IMG)
            G, j = divmod(g, XG)
            if j == 0:
                # one batched DMA covering XG transpose tiles: partition p is
                # position-within-tile, free dims (tile j, ci)
                n_in = min(XG, N_TILES - G * XG)
                xin = xin_pool.tile([TPOS, XG, CI], F32, name="xin", tag="xin")
                src0 = G * XG * TPOS
                nc.sync.dma_start(
                    out=xin[:, :n_in, :],
                    in_=x_flat[src0 : src0 + n_in * TPOS, :].rearrange(
                        "(j p) ci -> p j ci", j=n_in),
                )
                xin_groups[G] = xin
            xin = xin_groups[G]
            xc = xc_pool.tile([TPOS, CI], BF16, name="xc", tag="xc")
            copy_op(CFG["cast"], xc, xin[:, j, :])
            r0 = t * TROWS + 1  # padded row of first element
            for cc in range(CI_C):
                tps = tslot(g, cc)
                nc.tensor.transpose(
                    tps, xc[:, cc * P : (cc + 1) * P], ident[:TPOS, :TPOS]
                )
                tview = tps.rearrange("p (r x) -> p r x", x=W)
                hv = hi_plane[:, cc, b, r0 : r0 + TROWS, 1 : 1 + W]
                lv = lo_plane[:, cc, b, r0 : r0 + TROWS, 1 : 1 + W]
                copy_op(CFG["hi"], hv, tview)
                eng(CFG["lo"]).tensor_sub(lv, tview, hv)
        emitted[0] = max(emitted[0], min(N_TILES, upto))

    LOOKAHEAD = CFG["lookahead"]

    # First activation tiles go ahead of the weight load on the DMA pipe and
    # the DVE queue, so the PE transposes while the weights stream in.
    emit_transposes(LOOKAHEAD)
    emit_weights()

    for b in range(B):
        hiv = xt_hi[:, :, b * IMGP : (b + 1) * IMGP].rearrange(
            "p c (y x) -> p c y x", x=WP)
        lov = xt_lo[:, :, b * IMGP : (b + 1) * IMGP].rearrange(
            "p c (y x) -> p c y x", x=WP)
        for c in range(NCHUNK):
            y0 = c * YCHUNK
            # conv chunk c reads padded rows [y0, y0+10) = valid rows
            # [y0-1, y0+8] -> needs image tiles t < (y0+10)//2
            need = b * NT_IMG + min(NT_IMG, (y0 + YCHUNK + 2 + 1) // TROWS)
            emit_transposes(need + LOOKAHEAD)
            for oc in range(CO_C):
                cps = cpsum_pool.tile([P, FREE], F32, name="cps", tag="cps")
                n = 0
                for t in range(KK):
                    ky, kx = divmod(t, K)
                    lhs = s_all[:, t, oc, :]
                    planes = (hiv, lov) if t < K_LO else (hiv,)
                    for pl in planes:
                        rhs = pl[:, :, y0 + ky : y0 + ky + YCHUNK, kx : kx + W]
                        nc.tensor.matmul(
                            cps, lhs, rhs,
                            start=(n == 0), stop=(n == KK + K_LO - 1),
                            perf_mode=DRSW,
                        )
                        n += 1
                ot = out_pool.tile([P, FREE], F32, name="ot", tag="ot")
                copy_op(CFG["out"], ot, cps)
                outq = nc.scalar if CFG["outq"] == "act" else nc.sync
                outq.dma_start(
                    out=o_d.ap()[oc, :, b, y0 * W : (y0 + YCHUNK) * W],
                    in_=ot,
                )


def build_program(reps: int = 1):
    # Bacc (not plain Bass): compile() runs move_matmul_waits_to_ldweights +
    # generate_event_semaphores, required because walrus allows only one sync
    # wait per instruction.
    nc = bacc.Bacc("TRN2", debug=False, num_devices=N_CORES)
    x_d = nc.dram_tensor("x", [B, H, W, CI], F32, kind="ExternalInput")
    w_d = nc.dram_tensor("w", [K, K, CI, CO], F32, kind="ExternalInput")
    o_d = nc.dram_tensor("out", [CO_C, P, B, IMG], F32, kind="ExternalOutput")

    with tile.TileContext(nc) as tc:
        with (
            tc.tile_pool(name="const", bufs=1) as const_pool,
            tc.tile_pool(name="ws", bufs=1) as ws_pool,
            tc.tile_pool(name="win", bufs=1) as win_pool,
            tc.tile_pool(name="xin", bufs=(4 if CFG["xgroup"] > 1 else 12)) as xin_pool,
            tc.tile_pool(name="xcp", bufs=12) as xc_pool,
            tc.tile_pool(name="xtp", bufs=1) as xt_pool,
            tc.tile_pool(name="outs", bufs=4) as out_pool,
            tc.tile_pool(name="tpsum", bufs=(1 if CFG["tslot"] == "pack16" else 3), space="PSUM") as tpsum_pool,
            tc.tile_pool(name="cpsum", bufs=CFG["cpsum"], space="PSUM") as cpsum_pool,
        ):
            pools = (const_pool, ws_pool, win_pool, xin_pool, xc_pool,
                     xt_pool, out_pool, tpsum_pool, cpsum_pool)
            if reps == 1:
                _emit_body(nc, pools, x_d, w_d, o_d)
            else:
                with tc.For_i(0, reps, 1):
                    _emit_body(nc, pools, x_d, w_d, o_d)
    nc.compile()
    return nc


_NC_CACHE = {}


def _get_program(reps: int = 1):
    if reps not in _NC_CACHE:
        _NC_CACHE[reps] = build_program(reps)
    return _NC_CACHE[reps]


def kernel(x: np.ndarray, w: np.ndarray) -> np.ndarray:
    from concourse.bass_utils import run_bass_kernel_spmd

    x = np.ascontiguousarray(x, dtype=np.float32)
    w = np.ascontiguousarray(w, dtype=np.float32)
    nc = _get_program()
    in_maps = [
        {"x": np.ascontiguousarray(x[c * B : (c + 1) * B]), "w": w}
        for c in range(N_CORES)
    ]
    res = run_bass_kernel_spmd(nc, in_maps, core_ids=list(range(N_CORES))).results
    outs = []
    for c in range(N_CORES):
        r = res[c]["out"]  # (CO_C, P, B, IMG)
        r = r[:, ::-1]     # undo the DRSW column reversal within each oc chunk
        o = r.transpose(2, 3, 0, 1).reshape(B, H, W, CO)
        outs.append(o)
    return np.ascontiguousarray(np.concatenate(outs, axis=0))
